# revision 1
# baseline (speedup 1.0000x reference)
"""Trainium2 Bass kernel for a 3-layer GCN + 2-layer MLP (eval mode).

Math (per reference):
  src/dst = edge_index + self loops; deg over dst; dinv = rsqrt(max(deg,1))
  layer l: h = relu(BN_l(segsum_dst(norm * h[src]) @ W_l + b_l))
  out = relu(h @ lin_w1 + lin_b1) @ lin_w2 + lin_b2

Because aggregation is linear, each GCN layer is computed
aggregate-first:  h <- relu(BNaff(segsum(norm * h[src]) @ W')).
BN (eval) + conv bias fold into W' (column scale) and a shift row.

Distribution: nodes sharded contiguously over 8 cores (6250/core),
edges partitioned by destination.  The bf16 node-feature table
(pre-scaled by dinv[node], i.e. the source half of the GCN norm) is
replicated in HBM per core via AllGather between layers.  Each core
gathers its edges' source rows with the GPSIMD dma_gather custom DMA
(int16 indices => the source range is split in half; every call stays
under the ~1024-descriptor SWDGE ring capacity, round-robined over 4
SWDGE queues), then does the segment-sum as one-hot matmuls on the PE:
for each 128-edge block b of a 128-destination tile t,
    aggT[f, d] += M_b[e, f].T @ S_b[e, d].
The S_b one-hots are host-precomputed and streamed from HBM: layers
1-2 use exact fp8 0/1 matrices (dinv[dst] is applied via a
u=sqrt(deg)-scaled bias matmul plus a dinv^2 scale folded into the
ReLU that emits the next layer's table); layer 3 keeps its output
feature-major for the fused MLP, so its S carries dinv[dst] in bf16.

All GEMMs consume aggT (feature-major) directly as the matmul
stationary operand, so no transposes are needed anywhere; the final
MLP is computed feature-major and flipped back node-major by the last
matmul (lhsT = h4T).  Per-core output shards are concatenated on the
host.
"""

import sys

import numpy as np

sys.path.insert(0, "/opt/trn_rl_repo")

import ml_dtypes

# ---------------------------------------------------------------- config

CFG = dict(
    N=50000,       # nodes
    NC=8,          # cores
    P=128,
    NQUART=2,      # source-range splits (keeps int16 gather indices small;
                   # coarser splits waste fewer padded slots on the ceil-128
                   # block granularity)
    HID=128,
    OUT_C=40,
    BN_EPS=1e-5,
    MAXBLK=6,      # max 128-row blocks per dma_gather call (ring capacity)
    OWN=False,     # separate own-shard gather group (broke the scheduler)
)

TRACE = False          # set True to collect an NTFF profile
LAST_RESULTS = None    # BassKernelResults of the last kernel() call


# ---------------------------------------------------------- preprocessing

def _preprocess(edge_index, cfg):
    """Edge partitioning + per-core gather/one-hot metadata (numpy only)."""
    N, NC, P, NQ = cfg["N"], cfg["NC"], cfg["P"], cfg["NQUART"]
    NPC = N // NC
    TILES = (NPC + P - 1) // P
    QSZ = (N + NQ - 1) // NQ

    src = np.concatenate([edge_index[0], np.arange(N)]).astype(np.int64)
    dst = np.concatenate([edge_index[1], np.arange(N)]).astype(np.int64)

    deg = np.bincount(dst, minlength=N).astype(np.float32)
    dinv = (1.0 / np.sqrt(np.maximum(deg, 1.0))).astype(np.float32)

    core = dst // NPC
    ldst = dst - core * NPC
    tile = ldst // P
    dloc = ldst - tile * P
    src_core = src // NPC
    own = (src_core == core) & bool(cfg.get("OWN", False))
    quart = src // QSZ
    # group 0: source in this core's own shard (gathered from the local
    # pre-AllGather shard table, so these gathers overlap the collective);
    # groups 1..NQ: remaining sources by range quarter.
    NG = 1 + NQ
    grp = np.where(own, 0, 1 + quart)

    # B per (tile, group) = max over cores
    gid = (core * TILES + tile) * NG + grp
    counts = np.bincount(gid, minlength=NC * TILES * NG)
    per_ct = counts.reshape(NC, TILES * NG)
    B = np.ceil(per_ct.max(axis=0) / P).astype(np.int64).reshape(TILES, NG)
    B[:, 1:] = np.maximum(B[:, 1:], 1)

    # slot layout per core: tile-major, group runs inside each tile
    slot_q = np.zeros((TILES, NG), np.int64)
    off = 0
    for t in range(TILES):
        for g in range(NG):
            slot_q[t, g] = off
            off += B[t, g] * P
    NSLOT = int(off)
    NB = NSLOT // P

    # slot index per edge
    order = np.argsort(gid, kind="stable")
    gstart = np.zeros(NC * TILES * NG + 1, np.int64)
    np.cumsum(counts, out=gstart[1:])
    rank = np.arange(len(gid)) - gstart[gid[order]]
    t_s, g_s, c_s = tile[order], grp[order], core[order]
    flat = c_s * NSLOT + slot_q[t_s, g_s] + rank

    base = np.where(g_s == 0, c_s * NPC, (g_s - 1) * QSZ)
    gidx = np.zeros(NC * NSLOT, np.int16)
    gidx[flat] = (src[order] - base).astype(np.int16)

    # one-hot scatter matrices.  Layers 1-2 use a pure 0/1 one-hot in fp8
    # (exact; dinv[dst] is applied later via the u-bias + dinv^2 ReLU-scale
    # folding); layer 3's output stays feature-major, so its S carries
    # dinv[dst] directly in bf16.
    s8 = np.zeros((NC * NSLOT, P), ml_dtypes.float8_e4m3)
    s8[flat, dloc[order]] = 1.0
    sw = np.zeros((NC * NSLOT, P), ml_dtypes.bfloat16)
    sw[flat, dloc[order]] = dinv[dst[order]].astype(ml_dtypes.bfloat16)

    def wrap16(a):  # [NSLOT] -> [128, NSLOT//16]; slot i at [i%16, i//16], x8 replicated
        m = a.reshape(-1, 16).T
        return np.ascontiguousarray(np.tile(m, (8, 1)))

    def s_pack(a, dt):
        # [NSLOT, 128] -> flat [NSLOT*128] packed per tile: tile t's block
        # occupies a contiguous [128, nblk_t*128] region (partition-major)
        # so each per-tile DMA is one contiguous stream.
        out = np.empty(NSLOT * P, dt)
        for t in range(TILES):
            s0 = slot_q[t, 0]
            nblk = int(B[t].sum())
            blkv = a[s0:s0 + nblk * P].reshape(nblk, P, P)  # [b, p(slot), j]
            out[s0 * P:(s0 + nblk * P) * P] = (
                blkv.transpose(1, 0, 2).reshape(-1))      # [p, b, j] flat
        return out

    own_off = np.concatenate([[0], np.cumsum(B[:, 0])])
    meta = dict(
        NPC=NPC, TILES=TILES, NSLOT=NSLOT, NB=NB, QSZ=QSZ,
        B=B.tolist(), slot_q=slot_q.tolist(),
        own_off=own_off.tolist(), OWN_NB=int(own_off[-1]),
        gidx=[wrap16(gidx[c * NSLOT:(c + 1) * NSLOT]) for c in range(NC)],
        sdat8=[s_pack(s8[c * NSLOT:(c + 1) * NSLOT], ml_dtypes.float8_e4m3)
               for c in range(NC)],
        sdatw=[s_pack(sw[c * NSLOT:(c + 1) * NSLOT], ml_dtypes.bfloat16)
               for c in range(NC)],
    )

    # per-core per-tile node columns (pad rows -> 0):
    #   dinvloc [128, TILES]: dinv          (x prescale; layer-3 ReLU scale)
    #   dinvsq  [128, TILES]: dinv^2        (layer-1/2 ReLU scale)
    #   urows   [1, TILES*128]: sqrt(deg)   (layer-1/2 bias matmul lhsT)
    dinvloc, dinvsq, urows = [], [], []
    ids = np.arange(TILES * P)
    valid = ids < NPC
    u = np.sqrt(np.maximum(deg, 1.0)).astype(np.float32)
    for c in range(NC):
        fl = np.zeros(TILES * P, np.float32)
        fl[valid] = dinv[c * NPC + ids[valid]]
        dinvloc.append(np.ascontiguousarray(fl.reshape(TILES, P).T))
        dinvsq.append(np.ascontiguousarray((fl * fl).reshape(TILES, P).T))
        fu = np.zeros(TILES * P, np.float32)
        fu[valid] = u[c * NPC + ids[valid]]
        urows.append(np.ascontiguousarray(fu[None, :]))
    meta["dinvloc"] = dinvloc
    meta["dinvsq"] = dinvsq
    meta["urows"] = urows
    return meta


def _fold_weights(inp, cfg):
    eps = cfg["BN_EPS"]
    out = {}
    for i in (1, 2, 3):
        g, b = np.float32(inp[f"bn_g{i}"]), np.float32(inp[f"bn_b{i}"])
        m, v = np.float32(inp[f"bn_m{i}"]), np.float32(inp[f"bn_v{i}"])
        w, cb = np.float32(inp[f"conv_w{i}"]), np.float32(inp[f"conv_b{i}"])
        sc = g / np.sqrt(v + eps)
        out[f"wt{i}"] = np.ascontiguousarray(w * sc[None, :])
        out[f"sh{i}"] = np.ascontiguousarray(((cb - m) * sc + b)[None, :])
    out["w4"] = np.ascontiguousarray(np.float32(inp["lin_w1"]))
    out["b4"] = np.ascontiguousarray(np.float32(inp["lin_b1"])[None, :])
    out["w5"] = np.ascontiguousarray(np.float32(inp["lin_w2"]))
    out["b5"] = np.ascontiguousarray(np.float32(inp["lin_b2"])[None, :])
    return out


# ------------------------------------------------------------- bass build

def build_nc(meta, cfg):
    import concourse.bacc as bacc
    import concourse.mybir as mybir
    import concourse.tile as tile

    f32, bf16, i16 = mybir.dt.float32, mybir.dt.bfloat16, mybir.dt.int16
    Relu = mybir.ActivationFunctionType.Relu
    Copy = mybir.ActivationFunctionType.Copy
    BYP = mybir.AluOpType.bypass

    N, NC, P, NQ = cfg["N"], cfg["NC"], cfg["P"], cfg["NQUART"]
    OUT_C, MAXBLK = cfg["OUT_C"], cfg["MAXBLK"]
    NPC, TILES, NSLOT, NB = meta["NPC"], meta["TILES"], meta["NSLOT"], meta["NB"]
    QSZ = meta["QSZ"]
    B, slot_q = meta["B"], meta["slot_q"]
    own_off, OWN_NB = meta["own_off"], meta["OWN_NB"]

    nc = bacc.Bacc("TRN2", target_bir_lowering=False, debug=False,
                   num_devices=NC, num_swdge_queues=4)

    fp8 = mybir.dt.float8e4
    xs_t = nc.dram_tensor("xshard", [NPC, P], f32, kind="ExternalInput")
    gidx_t = nc.dram_tensor("gidx", [P, NSLOT // 16], i16, kind="ExternalInput")
    sdat8_t = nc.dram_tensor("sdat8", [NSLOT * P], fp8, kind="ExternalInput")
    sdatw_t = nc.dram_tensor("sdatw", [NSLOT * P], bf16, kind="ExternalInput")
    dinvloc_t = nc.dram_tensor("dinvloc", [P, TILES], f32, kind="ExternalInput")
    dinvsq_t = nc.dram_tensor("dinvsq", [P, TILES], f32, kind="ExternalInput")
    urows_t = nc.dram_tensor("urows", [1, TILES * P], f32, kind="ExternalInput")
    ones_t = nc.dram_tensor("onesr", [1, P], f32, kind="ExternalInput")
    wt1_t = nc.dram_tensor("wt1", [P, P], f32, kind="ExternalInput")
    sh1_t = nc.dram_tensor("sh1", [1, P], f32, kind="ExternalInput")
    wt2_t = nc.dram_tensor("wt2", [P, P], f32, kind="ExternalInput")
    sh2_t = nc.dram_tensor("sh2", [1, P], f32, kind="ExternalInput")
    wt3_t = nc.dram_tensor("wt3", [P, 2 * P], f32, kind="ExternalInput")
    sh3_t = nc.dram_tensor("sh3", [1, 2 * P], f32, kind="ExternalInput")
    w4_t = nc.dram_tensor("w4", [2 * P, P], f32, kind="ExternalInput")
    b4_t = nc.dram_tensor("b4", [1, P], f32, kind="ExternalInput")
    w5_t = nc.dram_tensor("w5", [P, OUT_C], f32, kind="ExternalInput")
    b5_t = nc.dram_tensor("b5", [1, OUT_C], f32, kind="ExternalInput")
    out_t = nc.dram_tensor("out", [NPC, OUT_C], f32, kind="ExternalOutput")

    t1s = nc.dram_tensor("t1s", [NPC, P], bf16)
    t1f = nc.dram_tensor("t1f", [N, P], bf16, addr_space="Shared")
    t2s = nc.dram_tensor("t2s", [NPC, P], bf16)
    t2f = nc.dram_tensor("t2f", [N, P], bf16, addr_space="Shared")
    t3s = nc.dram_tensor("t3s", [NPC, P], bf16)
    t3f = nc.dram_tensor("t3f", [N, P], bf16, addr_space="Shared")

    from contextlib import ExitStack

    with tile.TileContext(nc) as tc, ExitStack() as stk:
        const = stk.enter_context(tc.tile_pool(name="const", bufs=1))

        def load(t, shape, dt):
            sb = const.tile(shape, dt, tag=t.name)
            nc.sync.dma_start(sb[:], t[:])
            return sb

        gidx_sb = load(gidx_t, [P, NSLOT // 16], i16)
        dinvloc_sb = load(dinvloc_t, [P, TILES], f32)
        dinvsq_sb = load(dinvsq_t, [P, TILES], f32)
        u_sb = load(urows_t, [1, TILES * P], f32)
        ones_sb = load(ones_t, [1, P], f32)
        wt1_sb = load(wt1_t, [P, P], f32)
        sh1_sb = load(sh1_t, [1, P], f32)
        wt2_sb = load(wt2_t, [P, P], f32)
        sh2_sb = load(sh2_t, [1, P], f32)
        wt3_sb = load(wt3_t, [P, 2 * P], f32)
        sh3_sb = load(sh3_t, [1, 2 * P], f32)
        w4a_sb = const.tile([P, P], f32, tag="w4a")
        nc.sync.dma_start(w4a_sb[:], w4_t[0:P, :])
        w4b_sb = const.tile([P, P], f32, tag="w4b")
        nc.sync.dma_start(w4b_sb[:], w4_t[P:2 * P, :])
        b4_sb = load(b4_t, [1, P], f32)
        w5_sb = load(w5_t, [P, OUT_C], f32)
        b5_sb = load(b5_t, [1, OUT_C], f32)

        ch_pool = stk.enter_context(tc.tile_pool(name="chp", bufs=16))
        own_pool = stk.enter_context(tc.tile_pool(name="ownp", bufs=52))
        s_pool = stk.enter_context(tc.tile_pool(name="spool", bufs=8))
        agg_pool = stk.enter_context(tc.tile_pool(name="aggp", bufs=4))
        h_pool = stk.enter_context(tc.tile_pool(name="hp", bufs=4))
        o_pool = stk.enter_context(tc.tile_pool(name="op", bufs=3))
        ps_agg = stk.enter_context(tc.tile_pool(name="psagg", bufs=3, space="PSUM"))
        ps_y = stk.enter_context(tc.tile_pool(name="psy", bufs=2, space="PSUM"))
        ps_y4 = stk.enter_context(tc.tile_pool(name="psy4", bufs=1, space="PSUM"))
        ps_y5 = stk.enter_context(tc.tile_pool(name="psy5", bufs=2, space="PSUM"))

        # stage 0: per-shard x * dinv[node] -> bf16 table, then AllGather
        for t in range(TILES):
            rows = NPC - t * P if t == TILES - 1 else P
            xt = h_pool.tile([P, P], f32, tag="xt")
            nc.sync.dma_start(xt[:rows, :], xs_t[t * P:t * P + rows, :])
            xs = h_pool.tile([P, P], bf16, tag="xs")
            nc.scalar.activation(xs[:], xt[:], Copy,
                                 scale=dinvloc_sb[:, t:t + 1])
            nc.sync.dma_start(t1s[t * P:t * P + rows, :], xs[:rows, :])
        nc.gpsimd.collective_compute(
            "AllGather", BYP, replica_groups=[list(range(NC))],
            ins=[t1s[:].opt()], outs=[t1f[:].opt()])

        qcounter = [0]

        for L in (1, 2, 3):
            dt_m = bf16
            table = {1: t1f, 2: t2f, 3: t3f}[L]
            shard = {1: t1s, 2: t2s, 3: t3s}[L]
            # group 0 gathers from the local (pre-AllGather) shard table —
            # those calls don't depend on the collective and fill its window
            g_aps = [shard[0:NPC, :]] + [
                table[q * QSZ: min((q + 1) * QSZ, N), :] for q in range(NQ)]

            # own-shard gathers for every tile up front, each into its own
            # small buffer: they depend only on the local shard table, so
            # they run while the AllGather flies
            own_tiles = [None] * TILES
            for t in range(TILES):
                nb = B[t][0]
                if nb == 0:
                    continue
                slot0 = slot_q[t][0]
                ob = own_pool.tile([P, nb * P], dt_m, tag="own")
                own_tiles[t] = ob
                nc.gpsimd.dma_gather(
                    ob[:].rearrange("p (b e) -> p b e", e=P),
                    g_aps[0],
                    gidx_sb[:, slot0 // 16: slot0 // 16 + nb * 8],
                    nb * P, nb * P, P,
                    queue_num=qcounter[0] % 4,
                )
                qcounter[0] += 1

            for t in range(TILES):
                # gather this tile's remaining blocks: one (or more) calls
                # per source range, each under the SWDGE ring's budget
                nblk = sum(B[t])
                nbq = nblk - B[t][0]
                ch = ch_pool.tile([P, nbq * P], dt_m, tag="ch")
                for g in range(1, len(g_aps)):
                    done = 0
                    ncall = -(-B[t][g] // MAXBLK)
                    while done < B[t][g]:
                        rem = B[t][g] - done
                        nb = -(-rem // ncall)
                        ncall -= 1
                        slot0 = slot_q[t][g] + done * P
                        o0 = (slot0 - slot_q[t][1]) // P
                        nc.gpsimd.dma_gather(
                            ch[:, o0 * P:(o0 + nb) * P].rearrange(
                                "p (b e) -> p b e", e=P),
                            g_aps[g],
                            gidx_sb[:, slot0 // 16: slot0 // 16 + nb * 8],
                            nb * P, nb * P, P,
                            queue_num=qcounter[0] % 4,
                        )
                        qcounter[0] += 1
                        done += nb

                s0 = slot_q[t][0]
                ps = ps_agg.tile([P, P], f32)
                # stream host-built S block from HBM (HWDGE path); layers
                # 1-2 use the fp8 pure one-hot, layer 3 the bf16 weighted one
                if L < 3:
                    st = s_pool.tile([P, nblk * P], fp8, tag="st8")
                    src_s = sdat8_t
                else:
                    st = s_pool.tile([P, nblk * P], bf16, tag="stw")
                    src_s = sdatw_t
                nc.sync.dma_start(
                    st[:],
                    src_s[s0 * P:(s0 + nblk * P) * P].rearrange(
                        "(p f) -> p f", p=P))
                nown = B[t][0]
                for b in range(nblk):
                    if b < nown:
                        mb = own_tiles[t][:, b * P:(b + 1) * P]
                    else:
                        mb = ch[:, (b - nown) * P:(b - nown + 1) * P]
                    nc.tensor.matmul(ps[:], mb,
                                     st[:, b * P:(b + 1) * P],
                                     start=(b == 0), stop=(b == nblk - 1))

                aggT = agg_pool.tile([P, P], f32, tag="aggT")
                nc.vector.tensor_copy(aggT[:], ps[:])
                rows = NPC - t * P if t == TILES - 1 else P

                if True:

                    if L < 3:
                        wt_sb, sh_sb, tsh = (
                            (wt1_sb, sh1_sb, t2s) if L == 1 else (wt2_sb, sh2_sb, t3s)
                        )
                        psy = ps_y.tile([P, P], f32)
                        nc.tensor.matmul(psy[:], aggT[:], wt_sb[:],
                                         start=True, stop=False)
                        # bias scaled by u=sqrt(deg): the ReLU scale below is
                        # dinv^2 (one dinv for this layer's aggregation, one
                        # for the next layer's source prescale)
                        nc.tensor.matmul(psy[:],
                                         u_sb[:1, t * P:(t + 1) * P],
                                         sh_sb[:1, :], start=False, stop=True)
                        ht = h_pool.tile([P, P], bf16, tag="ht")
                        nc.scalar.activation(ht[:], psy[:], Relu,
                                             scale=dinvsq_sb[:, t:t + 1])
                        nc.sync.dma_start(tsh[t * P:t * P + rows, :], ht[:rows, :])
                    else:
                        h3s = []
                        for hf in range(2):
                            psy = ps_y.tile([P, P], f32)
                            nc.tensor.matmul(psy[:], wt3_sb[:, hf * P:(hf + 1) * P],
                                             aggT[:], start=True, stop=False)
                            nc.tensor.matmul(psy[:], sh3_sb[:1, hf * P:(hf + 1) * P],
                                             ones_sb[:1, :], start=False, stop=True)
                            h3 = h_pool.tile([P, P], f32, tag=f"h3{hf}")
                            nc.scalar.activation(h3[:], psy[:], Relu)
                            h3s.append(h3)
                        ps4 = ps_y4.tile([P, P], f32)
                        nc.tensor.matmul(ps4[:], w4a_sb[:], h3s[0][:],
                                         start=True, stop=False)
                        nc.tensor.matmul(ps4[:], w4b_sb[:], h3s[1][:],
                                         start=False, stop=False)
                        nc.tensor.matmul(ps4[:], b4_sb[:1, :], ones_sb[:1, :],
                                         start=False, stop=True)
                        h4 = h_pool.tile([P, P], f32, tag="h4")
                        nc.scalar.activation(h4[:], ps4[:], Relu)
                        ps5 = ps_y5.tile([P, OUT_C], f32)
                        nc.tensor.matmul(ps5[:], h4[:], w5_sb[:],
                                         start=True, stop=False)
                        nc.tensor.matmul(ps5[:], ones_sb[:1, :], b5_sb[:1, :],
                                         start=False, stop=True)
                        ot = o_pool.tile([P, OUT_C], f32, tag="ot")
                        nc.vector.tensor_copy(ot[:], ps5[:])
                        nc.sync.dma_start(out_t[t * P:t * P + rows, :],
                                          ot[:rows, :])

            if L == 1:
                nc.gpsimd.collective_compute(
                    "AllGather", BYP, replica_groups=[list(range(NC))],
                    ins=[t2s[:].opt()], outs=[t2f[:].opt()])
            elif L == 2:
                nc.gpsimd.collective_compute(
                    "AllGather", BYP, replica_groups=[list(range(NC))],
                    ins=[t3s[:].opt()], outs=[t3f[:].opt()])

    nc.compile()
    return nc


def make_in_maps(x, meta, folded, cfg):
    NC, P = cfg["NC"], cfg["P"]
    NPC = meta["NPC"]
    common = dict(
        onesr=np.ones((1, P), np.float32),
        **folded,
    )
    x = np.ascontiguousarray(np.float32(x))
    maps = []
    for c in range(NC):
        m = dict(common)
        m["xshard"] = np.ascontiguousarray(x[c * NPC:(c + 1) * NPC])
        m["gidx"] = meta["gidx"][c]
        m["sdat8"] = meta["sdat8"][c]
        m["sdatw"] = meta["sdatw"][c]
        m["dinvloc"] = meta["dinvloc"][c]
        m["dinvsq"] = meta["dinvsq"][c]
        m["urows"] = meta["urows"][c]
        maps.append(m)
    return maps


# ------------------------------------------------------------------ entry

def kernel(**inputs):
    global LAST_RESULTS
    from concourse.bass_utils import run_bass_kernel_spmd

    cfg = CFG
    x = np.asarray(inputs["x"])
    ei = np.asarray(inputs["edge_index"]).astype(np.int64)

    meta = _preprocess(ei, cfg)
    folded = _fold_weights(inputs, cfg)
    nc = build_nc(meta, cfg)
    in_maps = make_in_maps(x, meta, folded, cfg)

    res = run_bass_kernel_spmd(nc, in_maps, core_ids=list(range(cfg["NC"])),
                               trace=TRACE)
    LAST_RESULTS = res
    out = np.concatenate([res.results[c]["out"] for c in range(cfg["NC"])], axis=0)
    return np.ascontiguousarray(out, dtype=np.float32)



# revision 3
# speedup vs baseline: 1.2192x; 1.2192x over previous
"""Trainium2 Bass kernel for a 3-layer GCN + 2-layer MLP (eval mode).

Math (per reference):
  src/dst = edge_index + self loops; deg over dst; dinv = rsqrt(max(deg,1))
  norm[e] = dinv[src_e] * dinv[dst_e]
  layer l: h = relu(BN_l(segsum_dst(norm * h[src]) @ W_l + b_l))
  out = relu(h @ lin_w1 + lin_b1) @ lin_w2 + lin_b2

BN (eval) + conv bias fold into W' (column scale) and a shift row.
The full GCN norm is folded into the one-hot scatter matrix S, so node
tables are stored unscaled in bf16.

Distribution: nodes sharded contiguously over 8 cores (6250/core),
edges partitioned by destination.  Layer 1's per-edge source gather is
precomputed ON THE HOST into a contiguous message stream M1 (the x
table is a static input), so layer 1 does zero on-device gathers and
needs no AllGather.  Layers 2/3 gather from a bf16 node table that is
AllGathered in TWO chunks (sources split at local row 3200) so the
second chunk's collective overlaps the first chunk's gathers.

The segment-sum is computed on the PE as one-hot matmuls:
  aggT[f, d] += M_b[e, f].T @ S_b[e, d]
with S_b built ON-CHIP by one DVE op per block:
  S_b[e, d] = (iota[d] == dloc[e]) * norm[e]
All GEMMs consume aggT (feature-major) as lhsT; layer 1/2 outputs are
node-major (bias via ones x sh matmul), layer 3 + MLP run feature-major
with per-partition ACT biases; the last matmul flips node-major.
"""

import sys

import numpy as np

sys.path.insert(0, "/opt/trn_rl_repo")

import ml_dtypes

# ---------------------------------------------------------------- config

CFG = dict(
    N=50000,       # nodes
    NC=8,          # cores
    P=128,
    HID=128,
    OUT_C=40,
    BN_EPS=1e-5,
    CH0=3200,      # local rows in AG chunk 0 (= 25 tiles); chunk 1 = rest
    MAXBLK=6,      # max 128-row blocks per dma_gather call
    NGRP=8,        # tile groups (gather/stream granularity + buffer reuse)
)

TRACE = False          # set True to collect an NTFF profile
LAST_RESULTS = None    # BassKernelResults of the last kernel() call

BF16 = ml_dtypes.bfloat16


# ---------------------------------------------------------- preprocessing

def _slot_layout(counts_ct, P):
    """counts_ct: [NC, TILES, NG] per-(core,tile,group) edge counts.
    Returns B [TILES, NG] blocks (max over cores), slot offsets, NSLOT."""
    NC, TILES, NG = counts_ct.shape
    B = np.maximum(np.ceil(counts_ct.max(axis=0) / P).astype(np.int64), 1)
    slot_q = np.zeros((TILES, NG), np.int64)
    off = 0
    for t in range(TILES):
        for g in range(NG):
            slot_q[t, g] = off
            off += B[t, g] * P
    return B, slot_q, int(off)


def _preprocess(x, edge_index, cfg):
    """Edge partitioning + per-core metadata (numpy only)."""
    N, NC, P = cfg["N"], cfg["NC"], cfg["P"]
    CH0 = cfg["CH0"]
    NPC = N // NC
    TILES = (NPC + P - 1) // P
    CH1 = NPC - CH0

    src = np.concatenate([edge_index[0], np.arange(N)]).astype(np.int64)
    dst = np.concatenate([edge_index[1], np.arange(N)]).astype(np.int64)

    deg = np.bincount(dst, minlength=N).astype(np.float32)
    dinv = (1.0 / np.sqrt(np.maximum(deg, 1.0))).astype(np.float32)
    norm = dinv[src] * dinv[dst]

    core = dst // NPC
    ldst = dst - core * NPC
    tile = ldst // P
    dloc = ldst - tile * P
    s_core = src // NPC
    s_loc = src - s_core * NPC

    xb = np.asarray(x, np.float32).astype(BF16)

    meta = dict(NPC=NPC, TILES=TILES, CH0=CH0, CH1=CH1)

    # ---- layer 1: single group per tile; M1 pre-expanded on host ----
    gid1 = core * TILES + tile
    cnt1 = np.bincount(gid1, minlength=NC * TILES).reshape(NC, TILES, 1)
    B1, slotq1, NSLOT1 = _slot_layout(cnt1, P)
    NB1 = NSLOT1 // P
    order = np.argsort(gid1, kind="stable")
    gstart = np.zeros(NC * TILES + 1, np.int64)
    np.cumsum(cnt1.reshape(-1), out=gstart[1:])
    rank = np.arange(len(gid1)) - gstart[gid1[order]]
    flat1 = core[order] * NSLOT1 + slotq1[tile[order], 0] + rank

    src1 = np.zeros(NC * NSLOT1, np.int64)          # pad -> row 0
    src1[flat1] = src[order]
    dloc1 = np.full(NC * NSLOT1, -1.0, np.float32)  # pad -> no match
    dloc1[flat1] = dloc[order]
    nrm1 = np.zeros(NC * NSLOT1, np.float32)
    nrm1[flat1] = norm[order]

    m1, dloc1c, nrm1c = [], [], []
    for c in range(NC):
        s = src1[c * NSLOT1:(c + 1) * NSLOT1]
        g = xb[s].reshape(NB1, P, P).transpose(1, 0, 2)   # [p, b, f]
        m1.append(np.ascontiguousarray(g.reshape(P, NB1 * P)))
        dloc1c.append(np.ascontiguousarray(
            dloc1[c * NSLOT1:(c + 1) * NSLOT1].reshape(NB1, P).T))
        nrm1c.append(np.ascontiguousarray(
            nrm1[c * NSLOT1:(c + 1) * NSLOT1].reshape(NB1, P).T))
    meta.update(B1=B1[:, 0].tolist(), slotq1=slotq1[:, 0].tolist(),
                NB1=NB1, NSLOT1=NSLOT1, m1=m1, dloc1=dloc1c, nrm1=nrm1c)

    # ---- layers 2/3: two source-chunk groups per tile ----
    grp = (s_loc >= CH0).astype(np.int64)
    gid2 = (core * TILES + tile) * 2 + grp
    cnt2 = np.bincount(gid2, minlength=NC * TILES * 2).reshape(NC, TILES, 2)
    B2, slotq2, NSLOT2 = _slot_layout(cnt2, P)
    NB2 = NSLOT2 // P
    order = np.argsort(gid2, kind="stable")
    gstart = np.zeros(NC * TILES * 2 + 1, np.int64)
    np.cumsum(cnt2.reshape(-1), out=gstart[1:])
    rank = np.arange(len(gid2)) - gstart[gid2[order]]
    flat2 = (core[order] * NSLOT2
             + slotq2[tile[order], grp[order]] + rank)

    # chunked-table position, relative to own chunk
    pos = np.where(grp == 0,
                   s_core * CH0 + s_loc,
                   s_core * CH1 + (s_loc - CH0))
    gidx2 = np.zeros(NC * NSLOT2, np.int16)
    gidx2[flat2] = pos[order].astype(np.int16)
    dloc2 = np.full(NC * NSLOT2, -1.0, np.float32)
    dloc2[flat2] = dloc[order]
    nrm2 = np.zeros(NC * NSLOT2, np.float32)
    nrm2[flat2] = norm[order]

    def wrap16(a):  # [NSLOT] -> [128, NSLOT//16]; slot i at [i%16, i//16]
        m = a.reshape(-1, 16).T
        return np.ascontiguousarray(np.tile(m, (8, 1)))

    meta.update(
        B2=B2.tolist(), slotq2=slotq2.tolist(), NB2=NB2, NSLOT2=NSLOT2,
        gidx2=[wrap16(gidx2[c * NSLOT2:(c + 1) * NSLOT2]) for c in range(NC)],
        dloc2=[np.ascontiguousarray(
            dloc2[c * NSLOT2:(c + 1) * NSLOT2].reshape(NB2, P).T)
            for c in range(NC)],
        nrm2=[np.ascontiguousarray(
            nrm2[c * NSLOT2:(c + 1) * NSLOT2].reshape(NB2, P).T)
            for c in range(NC)],
    )
    return meta


def _fold_weights(inp, cfg):
    eps = cfg["BN_EPS"]
    P = cfg["P"]
    out = {}
    for i in (1, 2, 3):
        g, b = np.float32(inp[f"bn_g{i}"]), np.float32(inp[f"bn_b{i}"])
        m, v = np.float32(inp[f"bn_m{i}"]), np.float32(inp[f"bn_v{i}"])
        w, cb = np.float32(inp[f"conv_w{i}"]), np.float32(inp[f"conv_b{i}"])
        sc = g / np.sqrt(v + eps)
        out[f"wt{i}"] = np.ascontiguousarray((w * sc[None, :]).astype(BF16))
        sh = ((cb - m) * sc + b).astype(np.float32)
        if i < 3:
            out[f"sh{i}"] = np.ascontiguousarray(sh[None, :].astype(BF16))
        else:
            out["sh3c"] = np.ascontiguousarray(sh.reshape(2, P).T)  # f32 cols
    out["w4"] = np.ascontiguousarray(np.float32(inp["lin_w1"]).astype(BF16))
    out["b4c"] = np.ascontiguousarray(np.float32(inp["lin_b1"])[:, None])
    out["w5"] = np.ascontiguousarray(np.float32(inp["lin_w2"]).astype(BF16))
    out["b5"] = np.ascontiguousarray(
        np.float32(inp["lin_b2"])[None, :].astype(BF16))
    out["onesr"] = np.ones((1, P), BF16)
    out["iota"] = np.ascontiguousarray(
        np.broadcast_to(np.arange(P, dtype=np.float32)[None, :], (P, P)))
    return out


# ------------------------------------------------------------- bass build

def build_nc(meta, cfg):
    import concourse.bacc as bacc
    import concourse.mybir as mybir
    import concourse.tile as tile

    f32, bf16, i16 = mybir.dt.float32, mybir.dt.bfloat16, mybir.dt.int16
    Relu = mybir.ActivationFunctionType.Relu
    BYP = mybir.AluOpType.bypass
    EQ = mybir.AluOpType.is_equal
    MUL = mybir.AluOpType.mult

    N, NC, P = cfg["N"], cfg["NC"], cfg["P"]
    OUT_C, MAXBLK, NGRP = cfg["OUT_C"], cfg["MAXBLK"], cfg["NGRP"]
    NPC, TILES = meta["NPC"], meta["TILES"]
    CH0, CH1 = meta["CH0"], meta["CH1"]
    B1, slotq1, NB1, NSLOT1 = meta["B1"], meta["slotq1"], meta["NB1"], meta["NSLOT1"]
    B2, slotq2, NB2, NSLOT2 = meta["B2"], meta["slotq2"], meta["NB2"], meta["NSLOT2"]

    # tile groups: chunk0 = tiles 0..24, chunk1 = tiles 25..48
    half = NGRP // 2
    t_chunk0 = CH0 // P                      # 25
    g0 = [list(r) for r in np.array_split(np.arange(t_chunk0), half)]
    g1 = [list(r) for r in np.array_split(np.arange(t_chunk0, TILES), half)]
    groups = g0 + g1

    nc = bacc.Bacc("TRN2", target_bir_lowering=False, debug=False,
                   num_devices=NC, num_swdge_queues=4)

    m1_t = nc.dram_tensor("m1", [P, NSLOT1], bf16, kind="ExternalInput")
    dloc1_t = nc.dram_tensor("dloc1", [P, NB1], f32, kind="ExternalInput")
    nrm1_t = nc.dram_tensor("nrm1", [P, NB1], f32, kind="ExternalInput")
    gidx2_t = nc.dram_tensor("gidx2", [P, NSLOT2 // 16], i16, kind="ExternalInput")
    dloc2_t = nc.dram_tensor("dloc2", [P, NB2], f32, kind="ExternalInput")
    nrm2_t = nc.dram_tensor("nrm2", [P, NB2], f32, kind="ExternalInput")
    iota_t = nc.dram_tensor("iota", [P, P], f32, kind="ExternalInput")
    ones_t = nc.dram_tensor("onesr", [1, P], bf16, kind="ExternalInput")
    wt1_t = nc.dram_tensor("wt1", [P, P], bf16, kind="ExternalInput")
    sh1_t = nc.dram_tensor("sh1", [1, P], bf16, kind="ExternalInput")
    wt2_t = nc.dram_tensor("wt2", [P, P], bf16, kind="ExternalInput")
    sh2_t = nc.dram_tensor("sh2", [1, P], bf16, kind="ExternalInput")
    wt3_t = nc.dram_tensor("wt3", [P, 2 * P], bf16, kind="ExternalInput")
    sh3c_t = nc.dram_tensor("sh3c", [P, 2], f32, kind="ExternalInput")
    w4_t = nc.dram_tensor("w4", [2 * P, P], bf16, kind="ExternalInput")
    b4c_t = nc.dram_tensor("b4c", [P, 1], f32, kind="ExternalInput")
    w5_t = nc.dram_tensor("w5", [P, OUT_C], bf16, kind="ExternalInput")
    b5_t = nc.dram_tensor("b5", [1, OUT_C], bf16, kind="ExternalInput")
    out_t = nc.dram_tensor("out", [NPC, OUT_C], f32, kind="ExternalOutput")

    # per-chunk shard buffers + gathered tables (separate tensors => clean deps)
    t2sA = nc.dram_tensor("t2sA", [CH0, P], bf16)
    t2sB = nc.dram_tensor("t2sB", [CH1, P], bf16)
    t2fA = nc.dram_tensor("t2fA", [NC * CH0, P], bf16, addr_space="Shared")
    t2fB = nc.dram_tensor("t2fB", [NC * CH1, P], bf16, addr_space="Shared")
    t3sA = nc.dram_tensor("t3sA", [CH0, P], bf16)
    t3sB = nc.dram_tensor("t3sB", [CH1, P], bf16)
    t3fA = nc.dram_tensor("t3fA", [NC * CH0, P], bf16, addr_space="Shared")
    t3fB = nc.dram_tensor("t3fB", [NC * CH1, P], bf16, addr_space="Shared")

    from contextlib import ExitStack

    with tile.TileContext(nc) as tc, ExitStack() as stk:
        const = stk.enter_context(tc.tile_pool(name="const", bufs=1))

        def load(t, shape, dt):
            sb = const.tile(shape, dt, tag=t.name)
            nc.sync.dma_start(sb[:], t[:])
            return sb

        dloc1_sb = load(dloc1_t, [P, NB1], f32)
        nrm1_sb = load(nrm1_t, [P, NB1], f32)
        gidx2_sb = load(gidx2_t, [P, NSLOT2 // 16], i16)
        dloc2_sb = load(dloc2_t, [P, NB2], f32)
        nrm2_sb = load(nrm2_t, [P, NB2], f32)
        iota_sb = load(iota_t, [P, P], f32)
        ones_sb = load(ones_t, [1, P], bf16)
        wt1_sb = load(wt1_t, [P, P], bf16)
        sh1_sb = load(sh1_t, [1, P], bf16)
        wt2_sb = load(wt2_t, [P, P], bf16)
        sh2_sb = load(sh2_t, [1, P], bf16)
        wt3_sb = load(wt3_t, [P, 2 * P], bf16)
        sh3c_sb = load(sh3c_t, [P, 2], f32)
        w4a_sb = const.tile([P, P], bf16, tag="w4a")
        nc.sync.dma_start(w4a_sb[:], w4_t[0:P, :])
        w4b_sb = const.tile([P, P], bf16, tag="w4b")
        nc.sync.dma_start(w4b_sb[:], w4_t[P:2 * P, :])
        b4c_sb = load(b4c_t, [P, 1], f32)
        w5_sb = load(w5_t, [P, OUT_C], bf16)
        b5_sb = load(b5_t, [1, OUT_C], bf16)

        ch_pool = stk.enter_context(tc.tile_pool(name="chp", bufs=3))
        s_pool = stk.enter_context(tc.tile_pool(name="spool", bufs=8))
        agg_pool = stk.enter_context(tc.tile_pool(name="aggp", bufs=4))
        h_pool = stk.enter_context(tc.tile_pool(name="hp", bufs=6))
        o_pool = stk.enter_context(tc.tile_pool(name="op", bufs=3))
        ps_agg = stk.enter_context(tc.tile_pool(name="psagg", bufs=3, space="PSUM"))
        ps_y = stk.enter_context(tc.tile_pool(name="psy", bufs=2, space="PSUM"))
        ps_y4 = stk.enter_context(tc.tile_pool(name="psy4", bufs=1, space="PSUM"))
        ps_y5 = stk.enter_context(tc.tile_pool(name="psy5", bufs=2, space="PSUM"))

        qcounter = [0]

        def gen_s(blk):
            """One-hot(dloc)*norm for global block `blk` of layer L (1|2)."""
            st = s_pool.tile([P, P], bf16, tag="st")
            dl = dloc1_sb if gen_s.L == 1 else dloc2_sb
            nr = nrm1_sb if gen_s.L == 1 else nrm2_sb
            nc.vector.tensor_scalar(st[:], iota_sb[:],
                                    dl[:, blk:blk + 1], nr[:, blk:blk + 1],
                                    EQ, MUL)
            return st

        def tile_tail(L, t, aggps):
            """Consume the finished aggregation PSUM for tile t of layer L."""
            rows = NPC - t * P if t == TILES - 1 else P
            aggT = agg_pool.tile([P, P], bf16, tag="aggT")
            nc.vector.tensor_copy(aggT[:], aggps[:])
            if L < 3:
                wt_sb, sh_sb = (wt1_sb, sh1_sb) if L == 1 else (wt2_sb, sh2_sb)
                sA, sB = (t2sA, t2sB) if L == 1 else (t3sA, t3sB)
                psy = ps_y.tile([P, P], f32)
                nc.tensor.matmul(psy[:], aggT[:], wt_sb[:],
                                 start=True, stop=False)
                nc.tensor.matmul(psy[:], ones_sb[:1, :], sh_sb[:1, :],
                                 start=False, stop=True)
                ht = h_pool.tile([P, P], bf16, tag="ht")
                nc.scalar.activation(ht[:], psy[:], Relu)
                if t < t_chunk0:
                    nc.sync.dma_start(sA[t * P:t * P + rows, :], ht[:rows, :])
                else:
                    r0 = t * P - CH0
                    nc.sync.dma_start(sB[r0:r0 + rows, :], ht[:rows, :])
            else:
                h3s = []
                for hf in range(2):
                    psy = ps_y.tile([P, P], f32)
                    nc.tensor.matmul(psy[:], wt3_sb[:, hf * P:(hf + 1) * P],
                                     aggT[:], start=True, stop=True)
                    h3 = h_pool.tile([P, P], bf16, tag=f"h3{hf}")
                    nc.scalar.activation(h3[:], psy[:], Relu,
                                         bias=sh3c_sb[:, hf:hf + 1])
                    h3s.append(h3)
                ps4 = ps_y4.tile([P, P], f32)
                nc.tensor.matmul(ps4[:], w4a_sb[:], h3s[0][:],
                                 start=True, stop=False)
                nc.tensor.matmul(ps4[:], w4b_sb[:], h3s[1][:],
                                 start=False, stop=True)
                h4 = h_pool.tile([P, P], bf16, tag="h4")
                nc.scalar.activation(h4[:], ps4[:], Relu, bias=b4c_sb[:, 0:1])
                ps5 = ps_y5.tile([P, OUT_C], f32)
                nc.tensor.matmul(ps5[:], h4[:], w5_sb[:],
                                 start=True, stop=False)
                nc.tensor.matmul(ps5[:], ones_sb[:1, :], b5_sb[:1, :],
                                 start=False, stop=True)
                ot = o_pool.tile([P, OUT_C], f32, tag="ot")
                nc.vector.tensor_copy(ot[:], ps5[:])
                nc.sync.dma_start(out_t[t * P:t * P + rows, :], ot[:rows, :])

        t_chunk0 = CH0 // P

        # ---------------- layer 1: host-pre-expanded messages ----------------
        gen_s.L = 1
        for gi, tl in enumerate(groups):
            s0 = slotq1[tl[0]]
            s1 = slotq1[tl[-1]] + B1[tl[-1]] * P
            ch = ch_pool.tile([P, (s1 - s0)], bf16, tag="ch")
            nc.sync.dma_start(ch[:], m1_t[:, s0:s1])
            for t in tl:
                nblk = B1[t]
                b0 = slotq1[t] // P
                ps = ps_agg.tile([P, P], f32)
                for b in range(nblk):
                    st = gen_s(b0 + b)
                    off = (slotq1[t] - s0) + b * P
                    nc.tensor.matmul(ps[:], ch[:, off:off + P], st[:],
                                     start=(b == 0), stop=(b == nblk - 1))
                tile_tail(1, t, ps)
            if gi == half - 1:
                nc.gpsimd.collective_compute(
                    "AllGather", BYP, replica_groups=[list(range(NC))],
                    ins=[t2sA[:].opt()], outs=[t2fA[:].opt()])
            elif gi == NGRP - 1:
                nc.gpsimd.collective_compute(
                    "AllGather", BYP, replica_groups=[list(range(NC))],
                    ins=[t2sB[:].opt()], outs=[t2fB[:].opt()])

        # ---------------- layers 2 and 3: gathered tables ----------------
        for L in (2, 3):
            gen_s.L = 2
            tfA, tfB = (t2fA, t2fB) if L == 2 else (t3fA, t3fB)
            g_aps = [tfA[:, :], tfB[:, :]]
            for gi, tl in enumerate(groups):
                s0 = slotq2[tl[0]][0]
                s1 = slotq2[tl[-1]][1] + B2[tl[-1]][1] * P
                ch = ch_pool.tile([P, (s1 - s0)], bf16, tag="ch")
                for t in tl:
                    for g in (0, 1):
                        done = 0
                        ncall = -(-B2[t][g] // MAXBLK)
                        while done < B2[t][g]:
                            rem = B2[t][g] - done
                            nb = -(-rem // ncall)
                            ncall -= 1
                            slot0 = slotq2[t][g] + done * P
                            o0 = slot0 - s0
                            nc.gpsimd.dma_gather(
                                ch[:, o0:o0 + nb * P].rearrange(
                                    "p (b e) -> p b e", e=P),
                                g_aps[g],
                                gidx2_sb[:, slot0 // 16:
                                         slot0 // 16 + nb * 8],
                                nb * P, nb * P, P,
                                queue_num=qcounter[0] % 4,
                            )
                            qcounter[0] += 1
                            done += nb
                for t in tl:
                    nblk = B2[t][0] + B2[t][1]
                    b0 = slotq2[t][0] // P
                    ps = ps_agg.tile([P, P], f32)
                    for b in range(nblk):
                        st = gen_s(b0 + b)
                        off = (slotq2[t][0] - s0) + b * P
                        nc.tensor.matmul(ps[:], ch[:, off:off + P], st[:],
                                         start=(b == 0), stop=(b == nblk - 1))
                    tile_tail(L, t, ps)
                if L == 2:
                    if gi == half - 1:
                        nc.gpsimd.collective_compute(
                            "AllGather", BYP, replica_groups=[list(range(NC))],
                            ins=[t3sA[:].opt()], outs=[t3fA[:].opt()])
                    elif gi == NGRP - 1:
                        nc.gpsimd.collective_compute(
                            "AllGather", BYP, replica_groups=[list(range(NC))],
                            ins=[t3sB[:].opt()], outs=[t3fB[:].opt()])

    nc.compile()
    return nc


def make_in_maps(meta, folded, cfg):
    NC = cfg["NC"]
    common = dict(folded)
    maps = []
    for c in range(NC):
        m = dict(common)
        m["m1"] = meta["m1"][c]
        m["dloc1"] = meta["dloc1"][c]
        m["nrm1"] = meta["nrm1"][c]
        m["gidx2"] = meta["gidx2"][c]
        m["dloc2"] = meta["dloc2"][c]
        m["nrm2"] = meta["nrm2"][c]
        maps.append(m)
    return maps


# ------------------------------------------------------------------ entry

def kernel(**inputs):
    global LAST_RESULTS
    from concourse.bass_utils import run_bass_kernel_spmd

    cfg = CFG
    x = np.asarray(inputs["x"])
    ei = np.asarray(inputs["edge_index"]).astype(np.int64)

    meta = _preprocess(x, ei, cfg)
    folded = _fold_weights(inputs, cfg)
    nc = build_nc(meta, cfg)
    in_maps = make_in_maps(meta, folded, cfg)

    res = run_bass_kernel_spmd(nc, in_maps, core_ids=list(range(cfg["NC"])),
                               trace=TRACE)
    LAST_RESULTS = res
    out = np.concatenate([res.results[c]["out"] for c in range(cfg["NC"])], axis=0)
    return np.ascontiguousarray(out, dtype=np.float32)


# revision 4
# speedup vs baseline: 1.3357x; 1.0956x over previous
"""Trainium2 Bass kernel for a 3-layer GCN + 2-layer MLP (eval mode).

Math (per reference):
  src/dst = edge_index + self loops; deg over dst; dinv = rsqrt(max(deg,1))
  norm[e] = dinv[src_e] * dinv[dst_e]
  layer l: h = relu(BN_l(segsum_dst(norm * h[src]) @ W_l + b_l))
  out = relu(h @ lin_w1 + lin_b1) @ lin_w2 + lin_b2

BN (eval) + conv bias fold into W' (column scale) and a shift row.  The
full GCN norm is folded into the one-hot scatter matrices S (host-built
bf16, streamed from HBM), so node tables are stored unscaled in bf16.

Distribution: nodes sharded contiguously over 8 cores (6250/core),
edges partitioned by destination.  Layer 1's per-edge source gather is
precomputed ON THE HOST into a contiguous message stream M1 (the x
table is a static input), so layer 1 does zero on-device gathers and
needs no AllGather.  Layers 2/3 gather from a bf16 node table that is
AllGathered in TWO chunks (split at local row 3200 = 25 tiles), each
chunk a separate DRAM tensor for clean dependencies.

Layers 2/3 run TWO PASSES over destination tiles (pass g = source
chunk g): pass 0 accumulates each tile's partial aggregation into an
SBUF buffer; pass 1 adds the second chunk's contribution and finishes
the tile (GEMM / MLP).  This keeps every dma_gather in pass order on
the GpSimd queue, so gathers for chunk 0 never queue behind a wait for
chunk 1's AllGather.  Slots are sorted by source id inside each
(tile, chunk) for HBM locality.

The segment-sum is computed on the PE as one-hot matmuls:
  aggT[f, d] += M_b[e, f].T @ S_b[e, d]
All GEMMs consume aggT (feature-major) as lhsT; layer 1/2 outputs are
node-major (bias via ones x sh matmul), layer 3 + MLP run feature-major
with per-partition ACT biases; the last matmul flips node-major.
"""

import sys

import numpy as np

sys.path.insert(0, "/opt/trn_rl_repo")

import ml_dtypes

# ---------------------------------------------------------------- config

CFG = dict(
    N=50000,       # nodes
    NC=8,          # cores
    P=128,
    HID=128,
    OUT_C=40,
    BN_EPS=1e-5,
    CH0=3200,      # local rows in AG chunk 0 (= 25 tiles); chunk 1 = rest
    MAXBLK=6,      # max 128-row blocks per dma_gather call
    NGRP1=16,      # layer-1 stream groups (8 per AG chunk)
    NGRP=8,        # layer-2/3 tile groups per pass (4 per AG chunk)
)

TRACE = False          # set True to collect an NTFF profile
LAST_RESULTS = None    # BassKernelResults of the last kernel() call

BF16 = ml_dtypes.bfloat16


# ---------------------------------------------------------- preprocessing

def _pack_pmajor(a, P):
    """[NSLOT, W] -> [P, NSLOT//P*W] with slot s at [s%P, (s//P)*W + :W]."""
    nb = a.shape[0] // P
    return np.ascontiguousarray(
        a.reshape(nb, P, a.shape[1]).transpose(1, 0, 2).reshape(P, -1))


def _preprocess(x, edge_index, cfg):
    """Edge partitioning + per-core metadata (numpy only)."""
    N, NC, P = cfg["N"], cfg["NC"], cfg["P"]
    CH0 = cfg["CH0"]
    NPC = N // NC
    TILES = (NPC + P - 1) // P
    CH1 = NPC - CH0

    src = np.concatenate([edge_index[0], np.arange(N)]).astype(np.int64)
    dst = np.concatenate([edge_index[1], np.arange(N)]).astype(np.int64)

    deg = np.bincount(dst, minlength=N).astype(np.float32)
    dinv = (1.0 / np.sqrt(np.maximum(deg, 1.0))).astype(np.float32)
    norm = dinv[src] * dinv[dst]

    core = dst // NPC
    ldst = dst - core * NPC
    tile = ldst // P
    dloc = ldst - tile * P
    s_core = src // NPC
    s_loc = src - s_core * NPC

    xb = np.asarray(x, np.float32).astype(BF16)
    meta = dict(NPC=NPC, TILES=TILES, CH0=CH0, CH1=CH1)

    def layout(gid, ngroups, order):
        """Slot layout for group ids 0..NC*ngroups-1 (core-major).
        Returns per-(group) blocks B (max over cores), slot offsets, NSLOT,
        and flat slot index per edge (order = within-group sort order)."""
        counts = np.bincount(gid, minlength=NC * ngroups).reshape(NC, ngroups)
        B = np.maximum(np.ceil(counts.max(axis=0) / P).astype(np.int64), 1)
        slotq = np.zeros(ngroups, np.int64)
        np.cumsum(B[:-1] * P, out=slotq[1:])
        NSLOT = int((B * P).sum())
        gstart = np.zeros(NC * ngroups + 1, np.int64)
        np.cumsum(counts.reshape(-1), out=gstart[1:])
        rank = np.arange(len(gid)) - gstart[gid[order]]
        g_in_core = gid[order] % ngroups
        flat = (gid[order] // ngroups) * NSLOT + slotq[g_in_core] + rank
        return B, slotq, NSLOT, flat

    # ---- layer 1: single group per tile; M1 + S1 pre-built on host ----
    gid1 = core * TILES + tile
    order1 = np.lexsort((src, gid1))
    B1, slotq1, NSLOT1, flat1 = layout(gid1, TILES, order1)
    NB1 = NSLOT1 // P

    src1 = np.zeros(NC * NSLOT1, np.int64)
    src1[flat1] = src[order1]
    s1 = np.zeros((NC * NSLOT1, P), BF16)
    s1[flat1, dloc[order1]] = norm[order1].astype(BF16)

    m1, s1dat = [], []
    for c in range(NC):
        m1.append(_pack_pmajor(
            np.asarray(xb[src1[c * NSLOT1:(c + 1) * NSLOT1]]), P))
        s1dat.append(_pack_pmajor(s1[c * NSLOT1:(c + 1) * NSLOT1], P))
    del s1
    meta.update(B1=B1.tolist(), slotq1=slotq1.tolist(),
                NB1=NB1, NSLOT1=NSLOT1, m1=m1, s1dat=s1dat)

    # ---- layers 2/3: two source-chunk passes, g-major slot layout ----
    grp = (s_loc >= CH0).astype(np.int64)
    gid2 = core * (2 * TILES) + grp * TILES + tile     # g-major within core
    order2 = np.lexsort((src, gid2))
    B2, slotq2, NSLOT2, flat2 = layout(gid2, 2 * TILES, order2)
    NB2 = NSLOT2 // P

    pos = np.where(grp == 0,
                   s_core * CH0 + s_loc,
                   s_core * CH1 + (s_loc - CH0))
    gidx2 = np.zeros(NC * NSLOT2, np.int16)
    gidx2[flat2] = pos[order2].astype(np.int16)
    s2 = np.zeros((NC * NSLOT2, P), BF16)
    s2[flat2, dloc[order2]] = norm[order2].astype(BF16)

    def wrap16(a):  # [NSLOT] -> [128, NSLOT//16]; slot i at [i%16, i//16]
        m = a.reshape(-1, 16).T
        return np.ascontiguousarray(np.tile(m, (8, 1)))

    meta.update(
        B2=B2.reshape(2, TILES).tolist(),
        slotq2=slotq2.reshape(2, TILES).tolist(),
        NB2=NB2, NSLOT2=NSLOT2,
        gidx2=[wrap16(gidx2[c * NSLOT2:(c + 1) * NSLOT2]) for c in range(NC)],
        s2dat=[_pack_pmajor(s2[c * NSLOT2:(c + 1) * NSLOT2], P)
               for c in range(NC)],
    )
    del s2
    return meta


def _fold_weights(inp, cfg):
    eps = cfg["BN_EPS"]
    P = cfg["P"]
    out = {}
    for i in (1, 2, 3):
        g, b = np.float32(inp[f"bn_g{i}"]), np.float32(inp[f"bn_b{i}"])
        m, v = np.float32(inp[f"bn_m{i}"]), np.float32(inp[f"bn_v{i}"])
        w, cb = np.float32(inp[f"conv_w{i}"]), np.float32(inp[f"conv_b{i}"])
        sc = g / np.sqrt(v + eps)
        out[f"wt{i}"] = np.ascontiguousarray((w * sc[None, :]).astype(BF16))
        sh = ((cb - m) * sc + b).astype(np.float32)
        if i < 3:
            out[f"sh{i}"] = np.ascontiguousarray(sh[None, :].astype(BF16))
        else:
            out["sh3c"] = np.ascontiguousarray(sh.reshape(2, P).T)  # f32 cols
    out["w4"] = np.ascontiguousarray(np.float32(inp["lin_w1"]).astype(BF16))
    out["b4c"] = np.ascontiguousarray(np.float32(inp["lin_b1"])[:, None])
    out["w5"] = np.ascontiguousarray(np.float32(inp["lin_w2"]).astype(BF16))
    out["b5"] = np.ascontiguousarray(
        np.float32(inp["lin_b2"])[None, :].astype(BF16))
    out["onesr"] = np.ones((1, P), BF16)
    return out


# ------------------------------------------------------------- bass build

def build_nc(meta, cfg):
    import concourse.bacc as bacc
    import concourse.mybir as mybir
    import concourse.tile as tile

    f32, bf16, i16 = mybir.dt.float32, mybir.dt.bfloat16, mybir.dt.int16
    Relu = mybir.ActivationFunctionType.Relu
    BYP = mybir.AluOpType.bypass
    ADD = mybir.AluOpType.add

    N, NC, P = cfg["N"], cfg["NC"], cfg["P"]
    OUT_C, MAXBLK = cfg["OUT_C"], cfg["MAXBLK"]
    NGRP1, NGRP = cfg["NGRP1"], cfg["NGRP"]
    NPC, TILES = meta["NPC"], meta["TILES"]
    CH0, CH1 = meta["CH0"], meta["CH1"]
    B1, slotq1, NB1, NSLOT1 = meta["B1"], meta["slotq1"], meta["NB1"], meta["NSLOT1"]
    B2, slotq2, NB2, NSLOT2 = meta["B2"], meta["slotq2"], meta["NB2"], meta["NSLOT2"]

    t_chunk0 = CH0 // P                      # 25

    def split(lo, hi, n):
        return [list(r) for r in np.array_split(np.arange(lo, hi), n)]

    groups1 = split(0, t_chunk0, NGRP1 // 2) + split(t_chunk0, TILES, NGRP1 // 2)
    groups2 = split(0, t_chunk0, NGRP // 2) + split(t_chunk0, TILES, NGRP // 2)

    nc = bacc.Bacc("TRN2", target_bir_lowering=False, debug=False,
                   num_devices=NC, num_swdge_queues=4)

    m1_t = nc.dram_tensor("m1", [P, NSLOT1], bf16, kind="ExternalInput")
    s1_t = nc.dram_tensor("s1dat", [P, NSLOT1], bf16, kind="ExternalInput")
    gidx2_t = nc.dram_tensor("gidx2", [P, NSLOT2 // 16], i16, kind="ExternalInput")
    s2_t = nc.dram_tensor("s2dat", [P, NSLOT2], bf16, kind="ExternalInput")
    ones_t = nc.dram_tensor("onesr", [1, P], bf16, kind="ExternalInput")
    wt1_t = nc.dram_tensor("wt1", [P, P], bf16, kind="ExternalInput")
    sh1_t = nc.dram_tensor("sh1", [1, P], bf16, kind="ExternalInput")
    wt2_t = nc.dram_tensor("wt2", [P, P], bf16, kind="ExternalInput")
    sh2_t = nc.dram_tensor("sh2", [1, P], bf16, kind="ExternalInput")
    wt3_t = nc.dram_tensor("wt3", [P, 2 * P], bf16, kind="ExternalInput")
    sh3c_t = nc.dram_tensor("sh3c", [P, 2], f32, kind="ExternalInput")
    w4_t = nc.dram_tensor("w4", [2 * P, P], bf16, kind="ExternalInput")
    b4c_t = nc.dram_tensor("b4c", [P, 1], f32, kind="ExternalInput")
    w5_t = nc.dram_tensor("w5", [P, OUT_C], bf16, kind="ExternalInput")
    b5_t = nc.dram_tensor("b5", [1, OUT_C], bf16, kind="ExternalInput")
    out_t = nc.dram_tensor("out", [NPC, OUT_C], f32, kind="ExternalOutput")

    # per-chunk shard buffers + gathered tables (separate tensors => clean deps)
    t2sA = nc.dram_tensor("t2sA", [CH0, P], bf16)
    t2sB = nc.dram_tensor("t2sB", [CH1, P], bf16)
    t2fA = nc.dram_tensor("t2fA", [NC * CH0, P], bf16, addr_space="Shared")
    t2fB = nc.dram_tensor("t2fB", [NC * CH1, P], bf16, addr_space="Shared")
    t3sA = nc.dram_tensor("t3sA", [CH0, P], bf16)
    t3sB = nc.dram_tensor("t3sB", [CH1, P], bf16)
    t3fA = nc.dram_tensor("t3fA", [NC * CH0, P], bf16, addr_space="Shared")
    t3fB = nc.dram_tensor("t3fB", [NC * CH1, P], bf16, addr_space="Shared")

    from contextlib import ExitStack

    with tile.TileContext(nc) as tc, ExitStack() as stk:
        const = stk.enter_context(tc.tile_pool(name="const", bufs=1))

        def load(t, shape, dt):
            sb = const.tile(shape, dt, tag=t.name)
            nc.sync.dma_start(sb[:], t[:])
            return sb

        gidx2_sb = load(gidx2_t, [P, NSLOT2 // 16], i16)
        ones_sb = load(ones_t, [1, P], bf16)
        wt1_sb = load(wt1_t, [P, P], bf16)
        sh1_sb = load(sh1_t, [1, P], bf16)
        wt2_sb = load(wt2_t, [P, P], bf16)
        sh2_sb = load(sh2_t, [1, P], bf16)
        wt3_sb = load(wt3_t, [P, 2 * P], bf16)
        sh3c_sb = load(sh3c_t, [P, 2], f32)
        w4a_sb = const.tile([P, P], bf16, tag="w4a")
        nc.sync.dma_start(w4a_sb[:], w4_t[0:P, :])
        w4b_sb = const.tile([P, P], bf16, tag="w4b")
        nc.sync.dma_start(w4b_sb[:], w4_t[P:2 * P, :])
        b4c_sb = load(b4c_t, [P, 1], f32)
        w5_sb = load(w5_t, [P, OUT_C], bf16)
        b5_sb = load(b5_t, [1, OUT_C], bf16)

        sg_pool = stk.enter_context(tc.tile_pool(name="sgp", bufs=4))
        ch_pool = stk.enter_context(tc.tile_pool(name="chp", bufs=6))
        acc_pool = stk.enter_context(tc.tile_pool(name="accp", bufs=1))
        agg_pool = stk.enter_context(tc.tile_pool(name="aggp", bufs=4))
        h_pool = stk.enter_context(tc.tile_pool(name="hp", bufs=6))
        o_pool = stk.enter_context(tc.tile_pool(name="op", bufs=3))
        ps_agg = stk.enter_context(tc.tile_pool(name="psagg", bufs=3, space="PSUM"))
        ps_y = stk.enter_context(tc.tile_pool(name="psy", bufs=2, space="PSUM"))
        ps_y4 = stk.enter_context(tc.tile_pool(name="psy4", bufs=1, space="PSUM"))
        ps_y5 = stk.enter_context(tc.tile_pool(name="psy5", bufs=2, space="PSUM"))

        qcounter = [0]

        def ag(src_ap, dst_ap):
            nc.gpsimd.collective_compute(
                "AllGather", BYP, replica_groups=[list(range(NC))],
                ins=[src_ap.opt()], outs=[dst_ap.opt()])

        def tile_tail(L, t, aggT):
            """GEMM/MLP + store for tile t of layer L, aggT [f,d] bf16."""
            rows = NPC - t * P if t == TILES - 1 else P
            if L < 3:
                wt_sb, sh_sb = (wt1_sb, sh1_sb) if L == 1 else (wt2_sb, sh2_sb)
                sA, sB = (t2sA, t2sB) if L == 1 else (t3sA, t3sB)
                psy = ps_y.tile([P, P], f32)
                nc.tensor.matmul(psy[:], aggT[:], wt_sb[:],
                                 start=True, stop=False)
                nc.tensor.matmul(psy[:], ones_sb[:1, :], sh_sb[:1, :],
                                 start=False, stop=True)
                ht = h_pool.tile([P, P], bf16, tag="ht")
                nc.scalar.activation(ht[:], psy[:], Relu)
                if t < t_chunk0:
                    nc.sync.dma_start(sA[t * P:t * P + rows, :], ht[:rows, :])
                else:
                    r0 = t * P - CH0
                    nc.sync.dma_start(sB[r0:r0 + rows, :], ht[:rows, :])
            else:
                h3s = []
                for hf in range(2):
                    psy = ps_y.tile([P, P], f32)
                    nc.tensor.matmul(psy[:], wt3_sb[:, hf * P:(hf + 1) * P],
                                     aggT[:], start=True, stop=True)
                    h3 = h_pool.tile([P, P], bf16, tag=f"h3{hf}")
                    nc.scalar.activation(h3[:], psy[:], Relu,
                                         bias=sh3c_sb[:, hf:hf + 1])
                    h3s.append(h3)
                ps4 = ps_y4.tile([P, P], f32)
                nc.tensor.matmul(ps4[:], w4a_sb[:], h3s[0][:],
                                 start=True, stop=False)
                nc.tensor.matmul(ps4[:], w4b_sb[:], h3s[1][:],
                                 start=False, stop=True)
                h4 = h_pool.tile([P, P], bf16, tag="h4")
                nc.scalar.activation(h4[:], ps4[:], Relu, bias=b4c_sb[:, 0:1])
                ps5 = ps_y5.tile([P, OUT_C], f32)
                nc.tensor.matmul(ps5[:], h4[:], w5_sb[:],
                                 start=True, stop=False)
                nc.tensor.matmul(ps5[:], ones_sb[:1, :], b5_sb[:1, :],
                                 start=False, stop=True)
                ot = o_pool.tile([P, OUT_C], f32, tag="ot")
                nc.vector.tensor_copy(ot[:], ps5[:])
                nc.sync.dma_start(out_t[t * P:t * P + rows, :], ot[:rows, :])

        # ---------------- layer 1: host-pre-expanded messages ----------------
        for gi, tl in enumerate(groups1):
            s0 = slotq1[tl[0]]
            s1e = slotq1[tl[-1]] + B1[tl[-1]] * P
            mg = sg_pool.tile([P, (s1e - s0)], bf16, tag="sg")
            nc.sync.dma_start(mg[:], m1_t[:, s0:s1e])
            sg = sg_pool.tile([P, (s1e - s0)], bf16, tag="sg")
            nc.sync.dma_start(sg[:], s1_t[:, s0:s1e])
            for t in tl:
                nblk = B1[t]
                off = slotq1[t] - s0
                ps = ps_agg.tile([P, P], f32)
                for b in range(nblk):
                    o = off + b * P
                    nc.tensor.matmul(ps[:], mg[:, o:o + P], sg[:, o:o + P],
                                     start=(b == 0), stop=(b == nblk - 1))
                aggT = agg_pool.tile([P, P], bf16, tag="aggT")
                nc.vector.tensor_copy(aggT[:], ps[:])
                tile_tail(1, t, aggT)
            if gi == NGRP1 // 2 - 1:
                ag(t2sA[:], t2fA[:])
            elif gi == NGRP1 - 1:
                ag(t2sB[:], t2fB[:])

        # ---------------- layers 2 and 3: two-pass gathered tables ----------
        for L in (2, 3):
            tfA, tfB = (t2fA, t2fB) if L == 2 else (t3fA, t3fB)
            g_aps = [tfA[:, :], tfB[:, :]]
            accv = acc_pool.tile([P, TILES * P], f32, tag="accv")
            for g in (0, 1):
                for gi, tl in enumerate(groups2):
                    s0 = slotq2[g][tl[0]]
                    s1e = slotq2[g][tl[-1]] + B2[g][tl[-1]] * P
                    sg = sg_pool.tile([P, (s1e - s0)], bf16, tag="sg")
                    nc.sync.dma_start(sg[:], s2_t[:, s0:s1e])
                    for t in tl:
                        nblk = B2[g][t]
                        ch = ch_pool.tile([P, nblk * P], bf16, tag="ch")
                        done = 0
                        ncall = -(-nblk // MAXBLK)
                        while done < nblk:
                            nb = -(-(nblk - done) // ncall)
                            ncall -= 1
                            slot0 = slotq2[g][t] + done * P
                            nc.gpsimd.dma_gather(
                                ch[:, done * P:(done + nb) * P].rearrange(
                                    "p (b e) -> p b e", e=P),
                                g_aps[g],
                                gidx2_sb[:, slot0 // 16:slot0 // 16 + nb * 8],
                                nb * P, nb * P, P,
                                queue_num=qcounter[0] % 4,
                            )
                            qcounter[0] += 1
                            done += nb
                        off = slotq2[g][t] - s0
                        ps = ps_agg.tile([P, P], f32)
                        for b in range(nblk):
                            nc.tensor.matmul(ps[:], ch[:, b * P:(b + 1) * P],
                                             sg[:, off + b * P:off + (b + 1) * P],
                                             start=(b == 0), stop=(b == nblk - 1))
                        if g == 0:
                            nc.vector.tensor_copy(accv[:, t * P:(t + 1) * P], ps[:])
                        else:
                            aggT = agg_pool.tile([P, P], bf16, tag="aggT")
                            nc.vector.tensor_tensor(
                                aggT[:], ps[:], accv[:, t * P:(t + 1) * P], ADD)
                            tile_tail(L, t, aggT)
                    if L == 2 and g == 1:
                        if gi == NGRP // 2 - 1:
                            ag(t3sA[:], t3fA[:])
                        elif gi == NGRP - 1:
                            ag(t3sB[:], t3fB[:])

    nc.compile()
    return nc


def make_in_maps(meta, folded, cfg):
    NC = cfg["NC"]
    maps = []
    for c in range(NC):
        m = dict(folded)
        m["m1"] = meta["m1"][c]
        m["s1dat"] = meta["s1dat"][c]
        m["gidx2"] = meta["gidx2"][c]
        m["s2dat"] = meta["s2dat"][c]
        maps.append(m)
    return maps


# ------------------------------------------------------------------ entry

def kernel(**inputs):
    global LAST_RESULTS
    from concourse.bass_utils import run_bass_kernel_spmd

    cfg = CFG
    x = np.asarray(inputs["x"])
    ei = np.asarray(inputs["edge_index"]).astype(np.int64)

    meta = _preprocess(x, ei, cfg)
    folded = _fold_weights(inputs, cfg)
    nc = build_nc(meta, cfg)
    in_maps = make_in_maps(meta, folded, cfg)

    res = run_bass_kernel_spmd(nc, in_maps, core_ids=list(range(cfg["NC"])),
                               trace=TRACE)
    LAST_RESULTS = res
    out = np.concatenate([res.results[c]["out"] for c in range(cfg["NC"])], axis=0)
    return np.ascontiguousarray(out, dtype=np.float32)
